# revision 1
# baseline (speedup 1.0000x reference)
"""CRF Viterbi decode (nn_CrfDecodeLayer) Trainium2 Bass kernel.

Problem: B=256, T=512, K=256 tags. Forward max-plus scan over T with
transition matrix trans[K,K], then backtrack to recover argmax tag path.
Output: tags [B, 514] int32 (padded to max_sequence_length + 2).

Sharding: data-parallel over batch: B=256 -> 8 cores x 32.

Per-core algorithm (B_loc=32, exact fp32, bit-identical to the jax ref):
  forward t=1..T-1:
    s_t[b,j] = max_i(s_{t-1}[b,i] + trans[i,j]) + em[b,t,j]
  layout: partitions p=(ic*32+b), ic=0..3 reduce i-subrange [64ic,64ic+64)
    64 fused scalar_tensor_tensor ops: acc = max(acc, trans_rep_k + s_col_k)
    combine: copy groups 1..3 down + 3 chained TT max -> m32 rows [32,256],
    fold [32,256]->[128,64], add folded emissions, store folded lattice.
  backtrack (recompute argmax instead of storing backpointers):
    tag_T-1 = argmax_j s_T-1[b,j]
    tag_t = argmax_i(s_t[b,i] + trans[i, tag_{t+1}])   (first-index ties)
    trans^T row gather via gpsimd indirect DMA fused with +s_t (compute_op=add);
    two independent batch-half chains interleaved to hide serial latency.
"""

import numpy as np

B, T, K = 256, 512, 256
NCORES = 8
BLOC = B // NCORES  # 32
OUT_T = T + 2  # 514
BIGN = float(T)  # iota shift: iota_neg = i - 512 (negative for all i < 512)
NCHAIN = 2  # backtrack chains (batch split)


def build_program(t_steps: int = T):
    """Build the SPMD Bass program (same program for all 8 cores)."""
    from contextlib import ExitStack

    import concourse.bass as bass
    import concourse.tile as tile
    from concourse import bacc, mybir

    FP32 = mybir.dt.float32
    INT32 = mybir.dt.int32
    A = mybir.AluOpType

    nc = bacc.Bacc("TRN2", target_bir_lowering=False, num_devices=NCORES)

    # ---- DRAM I/O ----
    # em_f[t, ic*32+b, k] = emissions[b, t, 64*ic+k]  (folded layout)
    em_f_d = nc.dram_tensor("em_f", [t_steps, 128, 64], FP32, kind="ExternalInput")
    trans_rep_d = nc.dram_tensor("trans_rep", [64, 128, K], FP32, kind="ExternalInput")
    transT_d = nc.dram_tensor("transT", [K, K], FP32, kind="ExternalInput")
    iota_neg_d = nc.dram_tensor("iota_neg", [BLOC, K], FP32, kind="ExternalInput")
    tags_d = nc.dram_tensor("tags", [BLOC, OUT_T], INT32, kind="ExternalOutput")
    # folded lattice: lat[t, ic*32+b, k] = s_t[b, 64*ic+k]
    lat_d = nc.dram_tensor("lat", [t_steps, 128, 64], FP32)

    with tile.TileContext(nc) as tc:
        with ExitStack() as ctx:
            static_pool = ctx.enter_context(tc.tile_pool(name="static", bufs=1))
            state_pool = ctx.enter_context(tc.tile_pool(name="state", bufs=3))
            pre_pool = ctx.enter_context(tc.tile_pool(name="pre", bufs=3))
            acc_pool = ctx.enter_context(tc.tile_pool(name="acc", bufs=2))
            em_pool = ctx.enter_context(tc.tile_pool(name="em", bufs=6))
            h_pool = ctx.enter_context(tc.tile_pool(name="h", bufs=2))
            row_pool = ctx.enter_context(tc.tile_pool(name="row", bufs=3))
            bt_pool = ctx.enter_context(tc.tile_pool(name="bt", bufs=12))
            sm_pool = ctx.enter_context(tc.tile_pool(name="sm", bufs=4))

            # ---- static loads ----
            trans_rep = static_pool.tile([128, 64, K], FP32)
            nc.sync.dma_start(trans_rep[:], trans_rep_d.ap().transpose([1, 0, 2]))
            iota_neg = static_pool.tile([BLOC, K], FP32)
            nc.sync.dma_start(iota_neg[:], iota_neg_d.ap())
            CHB = [(BLOC * c // NCHAIN, BLOC * (c + 1) // NCHAIN) for c in range(NCHAIN)]
            tags_fc = [
                static_pool.tile([hi - lo, T], FP32, name=f"tagsf{c}", tag=f"tagsf{c}")
                for c, (lo, hi) in enumerate(CHB)
            ]

            em_tiles = {}

            def em_load(t):
                if t >= t_steps:
                    return
                em_t = em_pool.tile([128, 64], FP32)
                nc.scalar.dma_start(em_t[:], em_f_d.ap()[t])
                em_tiles[t] = em_t

            # ---- t = 0: s_0 = em_0 ----
            s = state_pool.tile([128, 64], FP32)
            nc.sync.dma_start(s[:], em_f_d.ap()[0])
            nc.scalar.dma_start(lat_d.ap()[0], em_f_d.ap()[0])
            for t in (1, 2, 3):
                em_load(t)

            # ---- forward scan ----
            for t in range(1, t_steps):
                acc = acc_pool.tile([128, K], FP32)
                nc.vector.tensor_scalar(
                    acc[:], trans_rep[:, 0, :], s[:, 0:1], None, op0=A.add
                )
                em_load(t + 3)  # issued early; executes during the stt block
                for k in range(1, 64):
                    nc.vector.scalar_tensor_tensor(
                        acc[:], trans_rep[:, k, :], s[:, k : k + 1], acc[:],
                        op0=A.add, op1=A.max,
                    )
                # combine 4 partition-group partials: copy groups 1..3 down to
                # partitions 0-31, then chained TT max (starts on first arrival)
                g1 = h_pool.tile([BLOC, K], FP32, tag="g1")
                g2 = h_pool.tile([BLOC, K], FP32, tag="g2")
                g3 = h_pool.tile([BLOC, K], FP32, tag="g3")
                nc.sync.dma_start(g1[:], acc[BLOC : 2 * BLOC, :])
                nc.scalar.dma_start(g2[:], acc[2 * BLOC : 3 * BLOC, :])
                nc.sync.dma_start(g3[:], acc[3 * BLOC : 4 * BLOC, :])
                m32 = row_pool.tile([BLOC, K], FP32)
                nc.vector.tensor_tensor(out=m32[:], in0=acc[0:BLOC, :], in1=g1[:], op=A.max)
                nc.vector.tensor_tensor(out=m32[:], in0=m32[:], in1=g2[:], op=A.max)
                nc.vector.tensor_tensor(out=m32[:], in0=m32[:], in1=g3[:], op=A.max)
                # fold max-only rows by k-halves: state columns 0:32 become
                # ready first, so the next step's k<32 stt ops start while the
                # second half's fold DMAs are still in flight
                pre = pre_pool.tile([128, 64], FP32)
                em_t = em_tiles.pop(t)
                s = state_pool.tile([128, 64], FP32)
                for half in range(2):
                    kl, kh = half * 32, half * 32 + 32
                    for ic in range(4):
                        eng = nc.sync if ic % 2 == 0 else nc.scalar
                        eng.dma_start(
                            pre[ic * BLOC : (ic + 1) * BLOC, kl:kh],
                            m32[:, ic * 64 + kl : ic * 64 + kh],
                        )
                    nc.vector.tensor_tensor(
                        out=s[:, kl:kh], in0=pre[:, kl:kh], in1=em_t[:, kl:kh], op=A.add
                    )
                nc.scalar.dma_start(lat_d.ap()[t], s[:])

            # ---- backtrack: NCHAIN independent batch-slice chains ----
            def lat_rows(t, lo, hi):
                # [hi-lo, K] row view of folded lat[t]:
                # dst[b, 64*ic+k] = lat[t, ic*32 + lo + b, k]
                return lat_d.ap()[t].rearrange("(ic bb) k -> bb ic k", ic=4)[lo:hi]

            def argmax_step(val, t_col, c):
                nb = CHB[c][1] - CHB[c][0]
                m = sm_pool.tile([nb, 1], FP32, name=f"am{c}", tag=f"m{c}")
                nc.vector.tensor_reduce(m[:], val[:], axis=mybir.AxisListType.X, op=A.max)
                d = sm_pool.tile([nb, K], FP32, name=f"ad{c}", tag=f"d{c}")
                nc.vector.scalar_tensor_tensor(
                    d[:], val[:], m[:], iota_neg[0:nb, :], op0=A.is_ge, op1=A.mult
                )
                dmin = sm_pool.tile([nb, 1], FP32, name=f"admin{c}", tag=f"dmin{c}")
                nc.vector.tensor_reduce(dmin[:], d[:], axis=mybir.AxisListType.X, op=A.min)
                nc.scalar.copy(tags_fc[c][:, t_col : t_col + 1], dmin[:])
                idx = sm_pool.tile([nb, 1], INT32, name=f"aidx{c}", tag=f"idx{c}")
                nc.vector.tensor_scalar(idx[:], dmin[:], BIGN, None, op0=A.add)
                return idx

            idxs = [None] * NCHAIN
            for c, (lo, hi) in enumerate(CHB):
                sv = bt_pool.tile([hi - lo, K], FP32, name=f"sv{c}", tag=f"sv{c}")
                nc.sync.dma_start(sv[:], lat_rows(t_steps - 1, lo, hi))
                idxs[c] = argmax_step(sv, t_steps - 1, c)

            for t in range(t_steps - 2, -1, -1):
                svs = []
                for c, (lo, hi) in enumerate(CHB):
                    sv = bt_pool.tile([hi - lo, K], FP32, name=f"svl{c}", tag=f"sv{c}")
                    eng = nc.sync if c % 2 == 0 else nc.scalar
                    eng.dma_start(sv[:], lat_rows(t, lo, hi))
                    nc.gpsimd.indirect_dma_start(
                        out=sv[:],
                        out_offset=None,
                        in_=transT_d.ap(),
                        in_offset=bass.IndirectOffsetOnAxis(ap=idxs[c][:, :1], axis=0),
                        compute_op=A.add,
                    )
                    svs.append(sv)
                for c in range(NCHAIN):
                    idxs[c] = argmax_step(svs[c], t, c)

            # ---- output assembly (per chain; DMA merges partition offsets) ----
            for c, (lo, hi) in enumerate(CHB):
                tags_i = static_pool.tile(
                    [hi - lo, OUT_T], INT32, name=f"tagsi{c}", tag=f"tagsi{c}"
                )
                nc.vector.memset(tags_i[:], 0)
                nc.vector.tensor_scalar(
                    tags_i[:, 0:t_steps], tags_fc[c][:, 0:t_steps], BIGN, None, op0=A.add
                )
                nc.sync.dma_start(tags_d.ap()[lo:hi, :], tags_i[:])

    nc.compile()
    return nc


def _prep_inputs(emissions, transitions, t_steps: int = T):
    """Host-side layout prep. Returns per-core list of input dicts."""
    emissions = np.ascontiguousarray(emissions[:, :t_steps, :], dtype=np.float32)
    transitions = np.ascontiguousarray(transitions, dtype=np.float32)

    # trans_rep[k, ic*32+b, j] = trans[64*ic+k, j]
    tr = transitions.reshape(4, 64, K).transpose(1, 0, 2)  # [64, 4, K]
    trans_rep = np.broadcast_to(tr[:, :, None, :], (64, 4, BLOC, K)).reshape(64, 128, K)
    trans_rep = np.ascontiguousarray(trans_rep)
    transT = np.ascontiguousarray(transitions.T)
    iota_neg = np.ascontiguousarray(
        np.broadcast_to((np.arange(K, dtype=np.float32) - BIGN)[None, :], (BLOC, K))
    )

    in_maps = []
    for c in range(NCORES):
        em_c = emissions[c * BLOC : (c + 1) * BLOC]  # [32, t, K]
        # em_f[t, ic*32+b, k] = em_c[b, t, 64*ic+k]
        em_f = np.ascontiguousarray(
            em_c.reshape(BLOC, t_steps, 4, 64)
            .transpose(1, 2, 0, 3)
            .reshape(t_steps, 128, 64)
        )
        in_maps.append(
            {
                "em_f": em_f,
                "trans_rep": trans_rep,
                "transT": transT,
                "iota_neg": iota_neg,
            }
        )
    return in_maps


def kernel(emissions, transitions, mask, max_sequence_length):
    from concourse.bass_utils import run_bass_kernel_spmd

    emissions = np.asarray(emissions)
    transitions = np.asarray(transitions)
    mask = np.asarray(mask)

    nc = build_program(T)
    in_maps = _prep_inputs(emissions, transitions, T)
    res = run_bass_kernel_spmd(nc, in_maps, list(range(NCORES)))
    tags = np.concatenate([res.results[c]["tags"] for c in range(NCORES)], axis=0)
    tags = tags.astype(np.int32)
    tags[:, :T] *= mask.astype(np.int32)
    return tags



# revision 9
# speedup vs baseline: 1.0499x; 1.0499x over previous
"""CRF Viterbi decode (nn_CrfDecodeLayer) Trainium2 Bass kernel.

Problem: B=256, T=512, K=256 tags. Forward max-plus scan over T with
transition matrix trans[K,K], then backtrack to recover argmax tag path.
Output: tags [B, 514] int32 (padded to max_sequence_length + 2).

Sharding: data-parallel over batch: B=256 -> 8 cores x 32.

Per-core algorithm (B_loc=32, exact fp32, bit-identical to the jax ref):
  forward t=1..T-1:
    s_t[b,j] = max_i(s_{t-1}[b,i] + trans[i,j]) + em[b,t,j]
  layout: partitions p=(ic*32+b), ic=0..3 reduce i-subrange [64ic,64ic+64)
    64 fused scalar_tensor_tensor ops: acc = max(acc, trans_rep_k + s_col_k)
    combine: copy groups 1..3 down + 3 chained TT max -> m32 rows [32,256],
    fold [32,256]->[128,64], add folded emissions, store folded lattice.
  backtrack (recompute argmax instead of storing backpointers):
    tag_T-1 = argmax_j s_T-1[b,j]
    tag_t = argmax_i(s_t[b,i] + trans[i, tag_{t+1}])   (first-index ties)
    trans^T row gather via gpsimd indirect DMA fused with +s_t (compute_op=add);
    two independent batch-half chains interleaved to hide serial latency.
"""

import numpy as np

B, T, K = 256, 512, 256
NCORES = 8
BLOC = B // NCORES  # 32
OUT_T = T + 2  # 514
BIGN = float(T)  # iota shift: iota_neg = i - 512 (negative for all i < 512)
NCHAIN = 2  # backtrack chains (batch split)


def build_program(t_steps: int = T):
    """Build the SPMD Bass program (same program for all 8 cores)."""
    from contextlib import ExitStack

    import concourse.bass as bass
    import concourse.tile as tile
    from concourse import bacc, mybir

    FP32 = mybir.dt.float32
    INT32 = mybir.dt.int32
    A = mybir.AluOpType

    nc = bacc.Bacc("TRN2", target_bir_lowering=False, num_devices=NCORES)

    # ---- DRAM I/O ----
    # em_f[t, ic*32+b, k] = emissions[b, t, 64*ic+k]  (folded layout)
    em_f_d = nc.dram_tensor("em_f", [t_steps, 128, 64], FP32, kind="ExternalInput")
    trans_rep_d = nc.dram_tensor("trans_rep", [64, 128, K], FP32, kind="ExternalInput")
    transT_d = nc.dram_tensor("transT", [K, K], FP32, kind="ExternalInput")
    iota_neg_d = nc.dram_tensor("iota_neg", [BLOC, K], FP32, kind="ExternalInput")
    eye32_d = nc.dram_tensor("eye32", [BLOC, BLOC], FP32, kind="ExternalInput")
    tags_d = nc.dram_tensor("tags", [BLOC, OUT_T], INT32, kind="ExternalOutput")
    # folded lattice: lat[t, ic*32+b, k] = s_t[b, 64*ic+k]
    lat_d = nc.dram_tensor("lat", [t_steps, 128, 64], FP32)

    with tile.TileContext(nc) as tc:
        with ExitStack() as ctx:
            static_pool = ctx.enter_context(tc.tile_pool(name="static", bufs=1))
            state_pool = ctx.enter_context(tc.tile_pool(name="state", bufs=3))
            pre_pool = ctx.enter_context(tc.tile_pool(name="pre", bufs=2, space="PSUM"))
            acc_pool = ctx.enter_context(tc.tile_pool(name="acc", bufs=2))
            em_pool = ctx.enter_context(tc.tile_pool(name="em", bufs=6))
            h_pool = ctx.enter_context(tc.tile_pool(name="h", bufs=2))
            row_pool = ctx.enter_context(tc.tile_pool(name="row", bufs=3))
            bt_pool = ctx.enter_context(tc.tile_pool(name="bt", bufs=12))
            sm_pool = ctx.enter_context(tc.tile_pool(name="sm", bufs=4))

            # ---- static loads ----
            trans_rep = static_pool.tile([128, 64, K], FP32)
            nc.sync.dma_start(trans_rep[:], trans_rep_d.ap().transpose([1, 0, 2]))
            iota_neg = static_pool.tile([BLOC, K], FP32)
            nc.sync.dma_start(iota_neg[:], iota_neg_d.ap())
            eye32 = static_pool.tile([BLOC, BLOC], FP32)
            nc.sync.dma_start(eye32[:], eye32_d.ap())
            CHB = [(BLOC * c // NCHAIN, BLOC * (c + 1) // NCHAIN) for c in range(NCHAIN)]
            tags_fc = [
                static_pool.tile([hi - lo, T], FP32, name=f"tagsf{c}", tag=f"tagsf{c}")
                for c, (lo, hi) in enumerate(CHB)
            ]

            em_tiles = {}

            def em_load(t):
                if t >= t_steps:
                    return
                em_t = em_pool.tile([128, 64], FP32)
                nc.scalar.dma_start(em_t[:], em_f_d.ap()[t])
                em_tiles[t] = em_t

            # ---- t = 0: s_0 = em_0 ----
            s = state_pool.tile([128, 64], FP32)
            nc.sync.dma_start(s[:], em_f_d.ap()[0])
            nc.scalar.dma_start(lat_d.ap()[0], em_f_d.ap()[0])
            for t in (1, 2, 3):
                em_load(t)

            # ---- forward scan ----
            for t in range(1, t_steps):
                acc = acc_pool.tile([128, K], FP32)
                nc.vector.tensor_scalar(
                    acc[:], trans_rep[:, 0, :], s[:, 0:1], None, op0=A.add
                )
                em_load(t + 3)  # issued early; executes during the stt block
                for k in range(1, 64):
                    nc.vector.scalar_tensor_tensor(
                        acc[:], trans_rep[:, k, :], s[:, k : k + 1], acc[:],
                        op0=A.add, op1=A.max,
                    )
                # combine 4 partition-group partials: copy groups 1..3 down to
                # partitions 0-31, then chained TT max (starts on first arrival)
                g1 = h_pool.tile([BLOC, K], FP32, tag="g1")
                g2 = h_pool.tile([BLOC, K], FP32, tag="g2")
                g3 = h_pool.tile([BLOC, K], FP32, tag="g3")
                nc.sync.dma_start(g1[:], acc[BLOC : 2 * BLOC, :])
                nc.scalar.dma_start(g2[:], acc[2 * BLOC : 3 * BLOC, :])
                nc.sync.dma_start(g3[:], acc[3 * BLOC : 4 * BLOC, :])
                m32 = row_pool.tile([BLOC, K], FP32)
                nc.vector.tensor_tensor(out=m32[:], in0=acc[0:BLOC, :], in1=g1[:], op=A.max)
                nc.vector.tensor_tensor(out=m32[:], in0=m32[:], in1=g2[:], op=A.max)
                nc.vector.tensor_tensor(out=m32[:], in0=m32[:], in1=g3[:], op=A.max)
                # fold max-only rows via PE: 4 tiny matmuls with an identity
                # stationary copy m32 col-blocks to partition blocks of a PSUM
                # tile (tile_position = (0, 32*ic) derived from AP bases), then
                # one DVE add folds in the emissions
                pre = pre_pool.tile([128, 64], FP32)
                em_t = em_tiles.pop(t)
                s = state_pool.tile([128, 64], FP32)
                for ic in range(4):
                    nc.tensor.matmul(
                        pre[ic * BLOC : (ic + 1) * BLOC, :],
                        eye32[:],
                        m32[:, ic * 64 : ic * 64 + 64],
                        start=True,
                        stop=True,
                        tile_position=(0, ic * BLOC),
                    )
                nc.vector.tensor_tensor(
                    out=s[:], in0=pre[:], in1=em_t[:], op=A.add
                )
                nc.scalar.dma_start(lat_d.ap()[t], s[:])

            # ---- backtrack: NCHAIN independent batch-slice chains ----
            def lat_rows(t, lo, hi):
                # [hi-lo, K] row view of folded lat[t]:
                # dst[b, 64*ic+k] = lat[t, ic*32 + lo + b, k]
                return lat_d.ap()[t].rearrange("(ic bb) k -> bb ic k", ic=4)[lo:hi]

            def argmax_step(val, t_col, c):
                nb = CHB[c][1] - CHB[c][0]
                m = sm_pool.tile([nb, 1], FP32, name=f"am{c}", tag=f"m{c}")
                nc.vector.tensor_reduce(m[:], val[:], axis=mybir.AxisListType.X, op=A.max)
                d = sm_pool.tile([nb, K], FP32, name=f"ad{c}", tag=f"d{c}")
                nc.vector.scalar_tensor_tensor(
                    d[:], val[:], m[:], iota_neg[0:nb, :], op0=A.is_ge, op1=A.mult
                )
                dmin = sm_pool.tile([nb, 1], FP32, name=f"admin{c}", tag=f"dmin{c}")
                nc.vector.tensor_reduce(dmin[:], d[:], axis=mybir.AxisListType.X, op=A.min)
                nc.scalar.copy(tags_fc[c][:, t_col : t_col + 1], dmin[:])
                idx = sm_pool.tile([nb, 1], INT32, name=f"aidx{c}", tag=f"idx{c}")
                nc.vector.tensor_scalar(idx[:], dmin[:], BIGN, None, op0=A.add)
                return idx

            idxs = [None] * NCHAIN
            for c, (lo, hi) in enumerate(CHB):
                sv = bt_pool.tile([hi - lo, K], FP32, name=f"sv{c}", tag=f"sv{c}")
                nc.sync.dma_start(sv[:], lat_rows(t_steps - 1, lo, hi))
                idxs[c] = argmax_step(sv, t_steps - 1, c)

            for t in range(t_steps - 2, -1, -1):
                svs = []
                for c, (lo, hi) in enumerate(CHB):
                    sv = bt_pool.tile([hi - lo, K], FP32, name=f"svl{c}", tag=f"sv{c}")
                    eng = nc.sync if c % 2 == 0 else nc.scalar
                    eng.dma_start(sv[:], lat_rows(t, lo, hi))
                    nc.gpsimd.indirect_dma_start(
                        out=sv[:],
                        out_offset=None,
                        in_=transT_d.ap(),
                        in_offset=bass.IndirectOffsetOnAxis(ap=idxs[c][:, :1], axis=0),
                        compute_op=A.add,
                    )
                    svs.append(sv)
                for c in range(NCHAIN):
                    idxs[c] = argmax_step(svs[c], t, c)

            # ---- output assembly (per chain; DMA merges partition offsets) ----
            for c, (lo, hi) in enumerate(CHB):
                tags_i = static_pool.tile(
                    [hi - lo, OUT_T], INT32, name=f"tagsi{c}", tag=f"tagsi{c}"
                )
                nc.vector.memset(tags_i[:], 0)
                nc.vector.tensor_scalar(
                    tags_i[:, 0:t_steps], tags_fc[c][:, 0:t_steps], BIGN, None, op0=A.add
                )
                nc.sync.dma_start(tags_d.ap()[lo:hi, :], tags_i[:])

    nc.compile()
    return nc


def _prep_inputs(emissions, transitions, t_steps: int = T):
    """Host-side layout prep. Returns per-core list of input dicts."""
    emissions = np.ascontiguousarray(emissions[:, :t_steps, :], dtype=np.float32)
    transitions = np.ascontiguousarray(transitions, dtype=np.float32)

    # trans_rep[k, ic*32+b, j] = trans[64*ic+k, j]
    tr = transitions.reshape(4, 64, K).transpose(1, 0, 2)  # [64, 4, K]
    trans_rep = np.broadcast_to(tr[:, :, None, :], (64, 4, BLOC, K)).reshape(64, 128, K)
    trans_rep = np.ascontiguousarray(trans_rep)
    transT = np.ascontiguousarray(transitions.T)
    iota_neg = np.ascontiguousarray(
        np.broadcast_to((np.arange(K, dtype=np.float32) - BIGN)[None, :], (BLOC, K))
    )
    eye32 = np.eye(BLOC, dtype=np.float32)

    in_maps = []
    for c in range(NCORES):
        em_c = emissions[c * BLOC : (c + 1) * BLOC]  # [32, t, K]
        # em_f[t, ic*32+b, k] = em_c[b, t, 64*ic+k]
        em_f = np.ascontiguousarray(
            em_c.reshape(BLOC, t_steps, 4, 64)
            .transpose(1, 2, 0, 3)
            .reshape(t_steps, 128, 64)
        )
        in_maps.append(
            {
                "em_f": em_f,
                "trans_rep": trans_rep,
                "transT": transT,
                "iota_neg": iota_neg,
                "eye32": eye32,
            }
        )
    return in_maps


def kernel(emissions, transitions, mask, max_sequence_length):
    from concourse.bass_utils import run_bass_kernel_spmd

    emissions = np.asarray(emissions)
    transitions = np.asarray(transitions)
    mask = np.asarray(mask)

    nc = build_program(T)
    in_maps = _prep_inputs(emissions, transitions, T)
    res = run_bass_kernel_spmd(nc, in_maps, list(range(NCORES)))
    tags = np.concatenate([res.results[c]["tags"] for c in range(NCORES)], axis=0)
    tags = tags.astype(np.int32)
    tags[:, :T] *= mask.astype(np.int32)
    return tags



# revision 13
# speedup vs baseline: 1.2676x; 1.2074x over previous
"""CRF Viterbi decode (nn_CrfDecodeLayer) Trainium2 Bass kernel.

Problem: B=256, T=512, K=256 tags. Forward max-plus scan over T with
transition matrix trans[K,K], then backtrack to recover argmax tag path.
Output: tags [B, 514] int32 (padded to max_sequence_length + 2).

Sharding: data-parallel over batch: B=256 -> 8 cores x 32.

Per-core algorithm (B_loc=32, exact fp32, bit-identical to the jax ref):
  forward t=1..T-1:
    s_t[b,j] = max_i(s_{t-1}[b,i] + trans[i,j]) + em[b,t,j]
  layout: partitions p=(ic*32+b), ic=0..3 reduce i-subrange [64ic,64ic+64)
    64 fused scalar_tensor_tensor ops: acc = max(acc, trans_rep_k + s_col_k)
    combine: copy groups 1..3 down + 3 chained TT max -> m32 rows [32,256],
    fold [32,256]->[128,64], add folded emissions, store folded lattice.
  backtrack (recompute argmax instead of storing backpointers):
    tag_T-1 = argmax_j s_T-1[b,j]
    tag_t = argmax_i(s_t[b,i] + trans[i, tag_{t+1}])   (first-index ties)
    trans^T row gather via gpsimd indirect DMA fused with +s_t (compute_op=add);
    two independent batch-half chains interleaved to hide serial latency.
"""

import numpy as np

B, T, K = 256, 512, 256
NCORES = 8
BLOC = B // NCORES  # 32
OUT_T = T + 2  # 514
BIGN = float(T)  # iota shift: iota_neg = i - 512 (negative for all i < 512)
NCHAIN = 2  # backtrack chains (batch split)


def build_program(t_steps: int = T):
    """Build the SPMD Bass program (same program for all 8 cores)."""
    from contextlib import ExitStack

    import concourse.bass as bass
    import concourse.tile as tile
    from concourse import bacc, mybir

    FP32 = mybir.dt.float32
    INT32 = mybir.dt.int32
    A = mybir.AluOpType

    nc = bacc.Bacc("TRN2", target_bir_lowering=False, num_devices=NCORES)

    # ---- DRAM I/O ----
    # em_f[t, ic*32+b, k] = emissions[b, t, 64*ic+k]  (folded layout)
    em_f_d = nc.dram_tensor("em_f", [t_steps, 128, 64], FP32, kind="ExternalInput")
    trans_rep_d = nc.dram_tensor("trans_rep", [64, 128, K], FP32, kind="ExternalInput")
    transT_d = nc.dram_tensor("transT", [K, K], FP32, kind="ExternalInput")
    iota_neg_d = nc.dram_tensor("iota_neg", [BLOC, K], FP32, kind="ExternalInput")
    eye32_d = nc.dram_tensor("eye32", [BLOC, BLOC], FP32, kind="ExternalInput")
    tags_d = nc.dram_tensor("tags", [BLOC, OUT_T], INT32, kind="ExternalOutput")
    # folded lattice: lat[t, ic*32+b, k] = s_t[b, 64*ic+k]
    lat_d = nc.dram_tensor("lat", [t_steps, 128, 64], FP32)

    with tile.TileContext(nc) as tc:
        with ExitStack() as ctx:
            static_pool = ctx.enter_context(tc.tile_pool(name="static", bufs=1))
            state_pool = ctx.enter_context(tc.tile_pool(name="state", bufs=3))
            pre_pool = ctx.enter_context(tc.tile_pool(name="pre", bufs=2, space="PSUM"))
            acc_pool = ctx.enter_context(tc.tile_pool(name="acc", bufs=2))
            acc2_pool = ctx.enter_context(tc.tile_pool(name="acc2", bufs=2))
            em_pool = ctx.enter_context(tc.tile_pool(name="em", bufs=6))
            h_pool = ctx.enter_context(tc.tile_pool(name="h", bufs=2))
            row_pool = ctx.enter_context(tc.tile_pool(name="row", bufs=3))
            bt_pool = ctx.enter_context(tc.tile_pool(name="bt", bufs=12))
            sm_pool = ctx.enter_context(tc.tile_pool(name="sm", bufs=4))

            # ---- static loads ----
            trans_rep = static_pool.tile([128, 64, K], FP32)
            nc.sync.dma_start(trans_rep[:], trans_rep_d.ap().transpose([1, 0, 2]))
            iota_neg = static_pool.tile([BLOC, K], FP32)
            nc.sync.dma_start(iota_neg[:], iota_neg_d.ap())
            eye32 = static_pool.tile([BLOC, BLOC], FP32)
            nc.sync.dma_start(eye32[:], eye32_d.ap())
            CHB = [(BLOC * c // NCHAIN, BLOC * (c + 1) // NCHAIN) for c in range(NCHAIN)]
            tags_fc = [
                static_pool.tile([hi - lo, T], FP32, name=f"tagsf{c}", tag=f"tagsf{c}")
                for c, (lo, hi) in enumerate(CHB)
            ]

            em_tiles = {}

            def em_load(t):
                if t >= t_steps:
                    return
                em_t = em_pool.tile([128, 64], FP32)
                nc.scalar.dma_start(em_t[:], em_f_d.ap()[t])
                em_tiles[t] = em_t

            # ---- t = 0: s_0 = em_0 ----
            s = state_pool.tile([128, 64], FP32)
            nc.sync.dma_start(s[:], em_f_d.ap()[0])
            nc.scalar.dma_start(lat_d.ap()[0], em_f_d.ap()[0])
            for t in (1, 2, 3):
                em_load(t)

            # ---- forward scan ----
            for t in range(1, t_steps):
                # two alternating accumulators: consecutive DVE stt ops are
                # independent, hiding any RMW turnaround bubble
                acc = acc_pool.tile([128, K], FP32)
                acc2 = acc2_pool.tile([128, K], FP32)
                nc.vector.tensor_scalar(
                    acc[:], trans_rep[:, 0, :], s[:, 0:1], None, op0=A.add
                )
                nc.vector.tensor_scalar(
                    acc2[:], trans_rep[:, 1, :], s[:, 1:2], None, op0=A.add
                )
                em_load(t + 3)  # issued early; executes during the stt block
                for k in range(2, 64):
                    a = acc if k % 2 == 0 else acc2
                    nc.vector.scalar_tensor_tensor(
                        a[:], trans_rep[:, k, :], s[:, k : k + 1], a[:],
                        op0=A.add, op1=A.max,
                    )
                nc.vector.tensor_tensor(out=acc[:], in0=acc[:], in1=acc2[:], op=A.max)
                # combine 4 partition-group partials: copy groups 1..3 down to
                # partitions 0-31, then chained TT max (starts on first arrival)
                g1 = h_pool.tile([BLOC, K], FP32, tag="g1")
                g2 = h_pool.tile([BLOC, K], FP32, tag="g2")
                g3 = h_pool.tile([BLOC, K], FP32, tag="g3")
                nc.sync.dma_start(g1[:], acc[BLOC : 2 * BLOC, :])
                nc.scalar.dma_start(g2[:], acc[2 * BLOC : 3 * BLOC, :])
                nc.sync.dma_start(g3[:], acc[3 * BLOC : 4 * BLOC, :])
                m32 = row_pool.tile([BLOC, K], FP32)
                nc.vector.tensor_tensor(out=m32[:], in0=acc[0:BLOC, :], in1=g1[:], op=A.max)
                nc.vector.tensor_tensor(out=m32[:], in0=m32[:], in1=g2[:], op=A.max)
                nc.vector.tensor_tensor(out=m32[:], in0=m32[:], in1=g3[:], op=A.max)
                # fold max-only rows via PE: 4 tiny matmuls with an identity
                # stationary copy m32 col-blocks to partition blocks of a PSUM
                # tile (tile_position = (0, 32*ic) derived from AP bases), then
                # one DVE add folds in the emissions
                pre = pre_pool.tile([128, 64], FP32)
                em_t = em_tiles.pop(t)
                s = state_pool.tile([128, 64], FP32)
                for ic in range(4):
                    nc.tensor.matmul(
                        pre[ic * BLOC : (ic + 1) * BLOC, :],
                        eye32[:],
                        m32[:, ic * 64 : ic * 64 + 64],
                        start=True,
                        stop=True,
                        tile_position=(0, ic * BLOC),
                    )
                nc.vector.tensor_tensor(
                    out=s[:], in0=pre[:], in1=em_t[:], op=A.add
                )
                nc.scalar.dma_start(lat_d.ap()[t], s[:])

            # ---- backtrack: NCHAIN independent batch-slice chains ----
            def lat_rows(t, lo, hi):
                # [hi-lo, K] row view of folded lat[t]:
                # dst[b, 64*ic+k] = lat[t, ic*32 + lo + b, k]
                return lat_d.ap()[t].rearrange("(ic bb) k -> bb ic k", ic=4)[lo:hi]

            def argmax_step(val, t_col, c):
                nb = CHB[c][1] - CHB[c][0]
                m = sm_pool.tile([nb, 1], FP32, name=f"am{c}", tag=f"m{c}")
                nc.vector.tensor_reduce(m[:], val[:], axis=mybir.AxisListType.X, op=A.max)
                d = sm_pool.tile([nb, K], FP32, name=f"ad{c}", tag=f"d{c}")
                nc.vector.scalar_tensor_tensor(
                    d[:], val[:], m[:], iota_neg[0:nb, :], op0=A.is_ge, op1=A.mult
                )
                dmin = sm_pool.tile([nb, 1], FP32, name=f"admin{c}", tag=f"dmin{c}")
                nc.vector.tensor_reduce(dmin[:], d[:], axis=mybir.AxisListType.X, op=A.min)
                nc.scalar.copy(tags_fc[c][:, t_col : t_col + 1], dmin[:])
                idx = sm_pool.tile([nb, 1], INT32, name=f"aidx{c}", tag=f"idx{c}")
                nc.vector.tensor_scalar(idx[:], dmin[:], BIGN, None, op0=A.add)
                return idx

            idxs = [None] * NCHAIN
            for c, (lo, hi) in enumerate(CHB):
                sv = bt_pool.tile([hi - lo, K], FP32, name=f"sv{c}", tag=f"sv{c}")
                nc.sync.dma_start(sv[:], lat_rows(t_steps - 1, lo, hi))
                idxs[c] = argmax_step(sv, t_steps - 1, c)

            for t in range(t_steps - 2, -1, -1):
                svs = []
                for c, (lo, hi) in enumerate(CHB):
                    sv = bt_pool.tile([hi - lo, K], FP32, name=f"svl{c}", tag=f"sv{c}")
                    eng = nc.sync if c % 2 == 0 else nc.scalar
                    eng.dma_start(sv[:], lat_rows(t, lo, hi))
                    nc.gpsimd.indirect_dma_start(
                        out=sv[:],
                        out_offset=None,
                        in_=transT_d.ap(),
                        in_offset=bass.IndirectOffsetOnAxis(ap=idxs[c][:, :1], axis=0),
                        compute_op=A.add,
                    )
                    svs.append(sv)
                for c in range(NCHAIN):
                    idxs[c] = argmax_step(svs[c], t, c)

            # ---- output assembly (per chain; DMA merges partition offsets) ----
            for c, (lo, hi) in enumerate(CHB):
                tags_i = static_pool.tile(
                    [hi - lo, OUT_T], INT32, name=f"tagsi{c}", tag=f"tagsi{c}"
                )
                nc.vector.memset(tags_i[:], 0)
                nc.vector.tensor_scalar(
                    tags_i[:, 0:t_steps], tags_fc[c][:, 0:t_steps], BIGN, None, op0=A.add
                )
                nc.sync.dma_start(tags_d.ap()[lo:hi, :], tags_i[:])

    nc.compile()
    return nc


def _prep_inputs(emissions, transitions, t_steps: int = T):
    """Host-side layout prep. Returns per-core list of input dicts."""
    emissions = np.ascontiguousarray(emissions[:, :t_steps, :], dtype=np.float32)
    transitions = np.ascontiguousarray(transitions, dtype=np.float32)

    # trans_rep[k, ic*32+b, j] = trans[64*ic+k, j]
    tr = transitions.reshape(4, 64, K).transpose(1, 0, 2)  # [64, 4, K]
    trans_rep = np.broadcast_to(tr[:, :, None, :], (64, 4, BLOC, K)).reshape(64, 128, K)
    trans_rep = np.ascontiguousarray(trans_rep)
    transT = np.ascontiguousarray(transitions.T)
    iota_neg = np.ascontiguousarray(
        np.broadcast_to((np.arange(K, dtype=np.float32) - BIGN)[None, :], (BLOC, K))
    )
    eye32 = np.eye(BLOC, dtype=np.float32)

    in_maps = []
    for c in range(NCORES):
        em_c = emissions[c * BLOC : (c + 1) * BLOC]  # [32, t, K]
        # em_f[t, ic*32+b, k] = em_c[b, t, 64*ic+k]
        em_f = np.ascontiguousarray(
            em_c.reshape(BLOC, t_steps, 4, 64)
            .transpose(1, 2, 0, 3)
            .reshape(t_steps, 128, 64)
        )
        in_maps.append(
            {
                "em_f": em_f,
                "trans_rep": trans_rep,
                "transT": transT,
                "iota_neg": iota_neg,
                "eye32": eye32,
            }
        )
    return in_maps


def kernel(emissions, transitions, mask, max_sequence_length):
    from concourse.bass_utils import run_bass_kernel_spmd

    emissions = np.asarray(emissions)
    transitions = np.asarray(transitions)
    mask = np.asarray(mask)

    nc = build_program(T)
    in_maps = _prep_inputs(emissions, transitions, T)
    res = run_bass_kernel_spmd(nc, in_maps, list(range(NCORES)))
    tags = np.concatenate([res.results[c]["tags"] for c in range(NCORES)], axis=0)
    tags = tags.astype(np.int32)
    tags[:, :T] *= mask.astype(np.int32)
    return tags

